# revision 1
# baseline (speedup 1.0000x reference)
"""Trainium2 Bass kernel for DendSeqNet (dendritic spiking net, T=64 steps).

v2 strategy:
  - Pure data-parallel over batch: 8 cores x 16 batch elements, no collectives.
  - fp16 GEMM (10-bit mantissa == TF32 grid; verified bit-exact spike
    decisions vs the f32 reference) with WEIGHTS RESIDENT in SBUF (16.8MB
    fp16), x streamed per chunk. This allows time-chunked GEMM (8 steps,
    N=128 moving cols) without re-streaming weights.
  - Pipeline: GEMM chunk i (PE) || hidden recurrence chunk i-1 (DVE+Pool) ||
    out-layer GEMM chunk i-2 (PE, fp16 block-diagonal [128,40] stationary) ||
    out-dendrite LIF chunk i-2 (Pool, [40,16] partition layout).
  - Host pre-filter: x~(t) = 0.8 x~(t-1) + x(t), shifted one step so the
    GEMM emits exactly the current the membrane update needs.
  - Host post-filter: the readout (iso 0.8-filter + vso 0.9-filter + channel
    sum) is linear in the out-dendrite spikes zq, so the device only emits
    zq [40, T, BS] and the host applies the [T,T] double-exponential kernel.
  - Engine budget: PE ~225us (the roofline for this net at TF32-precision),
    DVE ~150us, Pool ~80us, ACT ~50us; everything overlaps the GEMM except
    the last chunk's recurrence tail.
"""

import numpy as np
from contextlib import ExitStack

import concourse.bacc as bacc
import concourse.tile as tile
import concourse.mybir as mybir
from concourse.bass_utils import run_bass_kernel_spmd

F32 = mybir.dt.float32
F16 = mybir.dt.float16
OP = mybir.AluOpType

N_CORES = 8
T, B, FS2 = 64, 128, 4096
HC, SPL1, H1 = 2, 2048, 2048
OC, SPL2, OUT = 4, 512, 10
BS = B // N_CORES          # 16 batch rows per core
ROWS = T * BS              # 1024 GEMM cols per core
KT = SPL1 // 128           # 16 contraction tiles per channel
MT = H1 // 128             # 16 output tiles per channel
GT = HC * KT               # 32 feature tiles of xT
NT = HC * MT               # 32 dendrite tiles
ST = H1 // 128             # 16 somatic tiles
CH = 8                     # steps per chunk
NCH = T // CH              # 8 chunks
NCOL = CH * BS             # 128 moving cols per chunk
OLAG = 2                   # out-layer lag in chunks


def build_nc(repeat=1):
    nc = bacc.Bacc("TRN2", target_bir_lowering=False)

    xT = nc.dram_tensor("xT", [128, GT, ROWS], F16, kind="ExternalInput")
    wh = nc.dram_tensor("wh", [HC, MT, 128, KT, 128], F16,
                        kind="ExternalInput")
    w2 = nc.dram_tensor("w2", [ST, 128, 40], F16, kind="ExternalInput")
    outd = nc.dram_tensor("outd", [40, T, BS], F16, kind="ExternalOutput")

    dve = nc.vector
    gp = nc.gpsimd

    with tile.TileContext(nc) as tc:
      for _rep in range(repeat):
        with ExitStack() as ctx:
            persist = ctx.enter_context(tc.tile_pool(name="persist", bufs=1))
            xpool = ctx.enter_context(tc.tile_pool(name="xpool", bufs=2))
            curpool = ctx.enter_context(tc.tile_pool(name="curp", bufs=2))
            zbpool = ctx.enter_context(tc.tile_pool(name="zbp", bufs=3))
            zdpool = ctx.enter_context(tc.tile_pool(name="zdp", bufs=2))
            copool = ctx.enter_context(tc.tile_pool(name="cop", bufs=2))
            gpsum = ctx.enter_context(
                tc.tile_pool(name="gpsum", bufs=2, space="PSUM"))
            opsum = ctx.enter_context(
                tc.tile_pool(name="opsum", bufs=2, space="PSUM"))

            # ---- persistent SBUF ----
            wht = [persist.tile([128, KT, 128], F16, tag=f"w{cm}",
                                name=f"wht{cm}")
                   for cm in range(HC * MT)]
            w2s = persist.tile([128, ST, 40], F16, tag="w2s")
            md = persist.tile([128, NT, BS], F32, tag="md")    # dend m=10v
            us = persist.tile([128, ST, BS], F32, tag="us")    # soma m=10v
            ish = persist.tile([128, ST, BS], F32, tag="ish")  # soma current
            qo = persist.tile([40, BS], F32, tag="qo")         # outd m=10v
            ido = persist.tile([40, BS], F32, tag="ido")       # outd current
            zqb = persist.tile([40, T, BS], F16, tag="zqb")    # outd spikes

            # ---- startup DMAs (consumption order) + state init ----
            # x chunk 0 split over both HWDGE queues (SP + ACT) so the
            # first matmul is gated on ~3us, then weights stream on both
            # queues at the PE consumption pace.
            dve.memset(md[:], 0.0)
            dve.memset(us[:], 0.0)
            gp.memset(ish[:], 0.0)
            gp.memset(qo[:], 0.0)
            gp.memset(ido[:], 0.0)

            xs_tiles = []

            def fetch_x(ch):
                xa = xpool.tile([128, KT, NCOL], F16, tag="xa",
                                name=f"xa{ch}")
                xb = xpool.tile([128, KT, NCOL], F16, tag="xb",
                                name=f"xb{ch}")
                nc.sync.dma_start(
                    xa[:], xT[:, 0:KT, ch * NCOL:(ch + 1) * NCOL])
                nc.scalar.dma_start(
                    xb[:], xT[:, KT:GT, ch * NCOL:(ch + 1) * NCOL])
                xs_tiles.append((xa, xb))

            fetch_x(0)
            for m in range(MT):
                for c in range(HC):
                    eng = nc.sync if (2 * m + c) % 2 == 0 else nc.scalar
                    eng.dma_start(wht[c * MT + m][:], wh[c, m])
            nc.sync.dma_start(w2s[:], w2[:].rearrange("g p q -> p g q"))

            cur_tiles = {}
            zb_tiles = {}
            co_tiles = {}

            def emit_gemm(ch):
                """hidden GEMM for chunk ch -> cur tile in SBUF (f32)."""
                xa, xb = xs_tiles[ch]
                curt = curpool.tile([128, NT, NCOL], F32, tag="cur")
                cur_tiles[ch] = curt
                for m in range(MT):
                    for c in range(HC):
                        ps = gpsum.tile([128, NCOL], F32, tag="gps")
                        wt = wht[c * MT + m]
                        xs = xa if c == 0 else xb
                        for k in range(KT):
                            nc.tensor.matmul(
                                ps[:], wt[:, k, :], xs[:, k, :],
                                start=(k == 0), stop=(k == KT - 1))
                        nc.scalar.copy(curt[:, c * MT + m, :], ps[:])
                if ch + 1 < NCH:
                    fetch_x(ch + 1)

            def emit_outgemm(ch):
                """out-layer GEMM on chunk ch's somatic spikes (fp16)."""
                zb = zb_tiles[ch]
                ops = opsum.tile([40, NCOL], F32, tag="ops")
                for g in range(ST):
                    nc.tensor.matmul(
                        ops[:], w2s[:, g, :], zb[:, g, :],
                        start=(g == 0), stop=(g == ST - 1))
                cot = copool.tile([40, NCOL], F32, tag="cot")
                co_tiles[ch] = cot
                nc.scalar.copy(cot[:], ops[:])

            def emit_hidden(ch):
                """hidden recurrence for chunk ch (DVE + Pool)."""
                curt = cur_tiles[ch]
                zbt = zbpool.tile([128, ST, NCOL], F16, tag="zb")
                zb_tiles[ch] = zbt
                for tl in range(CH):
                    col = slice(tl * BS, (tl + 1) * BS)
                    # D1: m = 0.9 m + i_d(t-1)
                    dve.scalar_tensor_tensor(
                        md[:], md[:], 0.9, curt[:, :, col], OP.mult, OP.add)
                    # zd = (m > 10), both channels in one op
                    zdt = zdpool.tile([128, NT, BS], F32, tag="zd")
                    dve.tensor_scalar(zdt[:], md[:], 10.0, None, OP.is_gt)
                    # D3: dendrite reset m = (m<=10)*m
                    dve.scalar_tensor_tensor(
                        md[:], md[:], 10.0, md[:], OP.is_le, OP.mult)
                    # S4: u = 0.9 u + i_s(old)   [reads ish before its update]
                    dve.scalar_tensor_tensor(
                        us[:], us[:], 0.9, ish[:], OP.mult, OP.add)
                    # Pool: zsum = zd[c0] + zd[c1]
                    zs2 = zdpool.tile([128, ST, BS], F32, tag="zs2")
                    gp.tensor_tensor(
                        zs2[:], zdt[:, 0:MT, :], zdt[:, MT:NT, :], OP.add)
                    # S5: z_s = (u > 10) -> fp16 for the out GEMM
                    dve.tensor_scalar(
                        zbt[:, :, col], us[:], 10.0, None, OP.is_gt)
                    # S6: soma reset u = (u<=10)*u
                    dve.scalar_tensor_tensor(
                        us[:], us[:], 10.0, us[:], OP.is_le, OP.mult)
                    # i_s = 0.8 i_s + zsum  (DVE; Pool lacks fused STT)
                    dve.scalar_tensor_tensor(
                        ish[:], ish[:], 0.8, zs2[:], OP.mult, OP.add)

            def emit_outlayer(ch):
                """out-dendrite LIF for chunk ch on Pool ([40, BS] state).

                Pool has no fused scalar_tensor_tensor, so decays and resets
                are split into tensor_scalar + tensor_tensor pairs."""
                cot = co_tiles[ch]
                for tl in range(CH):
                    t = ch * CH + tl
                    col = slice(tl * BS, (tl + 1) * BS)
                    # V1: q = 0.9 q + ido(old)
                    gp.tensor_scalar(qo[:], qo[:], 0.9, None, OP.mult)
                    gp.tensor_tensor(qo[:], qo[:], ido[:], OP.add)
                    # O: ido = 0.8 ido + cur_o(t)
                    gp.tensor_scalar(ido[:], ido[:], 0.8, None, OP.mult)
                    gp.tensor_tensor(ido[:], ido[:], cot[:, col], OP.add)
                    # Z: zq(t) = (q > 10)
                    gp.tensor_scalar(
                        zqb[:, t, :], qo[:], 10.0, None, OP.is_gt)
                    # QR: q = (q<=10)*q via mask
                    qm = zdpool.tile([40, BS], F32, tag="qm")
                    gp.tensor_scalar(qm[:], qo[:], 10.0, None, OP.is_le)
                    gp.tensor_tensor(qo[:], qo[:], qm[:], OP.mult)

            # ---- main pipeline ----
            for ch in range(NCH):
                emit_gemm(ch)
                if ch >= OLAG:
                    emit_outgemm(ch - OLAG)
                emit_hidden(ch)
                if ch >= OLAG:
                    emit_outlayer(ch - OLAG)

            # ---- drain: out-layer for the last OLAG chunks ----
            for ch in range(NCH - OLAG, NCH):
                emit_outgemm(ch)
                emit_outlayer(ch)

            nc.sync.dma_start(outd[:].rearrange("p t b -> p (t b)"),
                              zqb[:].rearrange("p t b -> p (t b)"))

    nc.finalize()
    return nc


def prep_inputs(x, w_hidden, w_out):
    """Host-side shard + repack. Returns per-core input maps."""
    x = np.ascontiguousarray(x, dtype=np.float32)
    # synaptic pre-filter, shifted one step (slot t holds x~(t-1))
    xf = np.zeros((T + 1, B, FS2), np.float32)
    acc = np.zeros(x.shape[1:], np.float32)
    for t in range(T - 1):
        acc = acc * np.float32(0.8) + x[t]
        xf[t + 1] = acc
    xh = xf[:T].astype(np.float16)
    whh = np.asarray(w_hidden, np.float32).astype(np.float16)
    woh = np.asarray(w_out, np.float32).astype(np.float16)
    # w_hidden [HC, SPL1, H1] -> [HC, MT, 128p, KT, 128q]
    whp = np.ascontiguousarray(
        whh.reshape(HC, KT, 128, MT, 128).transpose(0, 3, 2, 1, 4))
    # w_out [OC, SPL2, OUT] -> dense block-diagonal [ST, 128, 40]
    w2 = np.zeros((ST, 128, 40), np.float16)
    for g in range(ST):
        for i in range(128):
            f = g * 128 + i
            c = f // SPL2
            w2[g, i, c * OUT:(c + 1) * OUT] = woh[c, f % SPL2, :]
    in_maps = []
    for i in range(N_CORES):
        xs_ = xh[:, i * BS:(i + 1) * BS, :]              # [T, BS, FS2]
        xt = np.ascontiguousarray(
            xs_.reshape(ROWS, FS2).T.reshape(GT, 128, ROWS).transpose(1, 0, 2))
        in_maps.append({"xT": xt, "wh": whp, "w2": w2})
    return in_maps


def _readout_kernel():
    """Kcomb[t, u]: vso(t) = sum_u Kcomb[t,u] * zq-count(u), the composed
    0.8-synapse / 0.9-membrane double filter of the LI readout."""
    Kc = np.zeros((T, T), np.float64)
    for t in range(T):
        for u in range(t):          # iso(s) for s in [u, t-1]
            s = np.arange(u, t)
            Kc[t, u] = 0.1 * np.sum(0.9 ** (t - 1 - s) * 0.8 ** (s - u))
    return Kc.astype(np.float32)


_KCOMB = _readout_kernel()
_NC_CACHE = {}


def get_nc(repeat=1):
    if repeat not in _NC_CACHE:
        _NC_CACHE[repeat] = build_nc(repeat)
    return _NC_CACHE[repeat]


def run(inputs, trace=False, repeat=1, **kw):
    """Returns (full_output [T,B,10], BassKernelResults)."""
    nc = get_nc(repeat)
    in_maps = prep_inputs(inputs["x"], inputs["w_hidden"], inputs["w_out"])
    res = run_bass_kernel_spmd(nc, in_maps, list(range(N_CORES)),
                               trace=trace, **kw)
    out = np.empty((T, B, OUT), dtype=np.float32)
    for i in range(N_CORES):
        zq = np.asarray(res.results[i]["outd"]).astype(np.float32)
        zq = zq.reshape(OC, OUT, T, BS)
        # out[t, b, o] = sum_c sum_u Kcomb[t,u] zq[c, o, u, b]
        v = np.einsum('tu,oub->tbo', _KCOMB, zq.sum(0), optimize=True)
        out[:, i * BS:(i + 1) * BS, :] = v
    return out, res


def kernel(x, w_hidden, w_out):
    out, _ = run({"x": x, "w_hidden": w_hidden, "w_out": w_out})
    return out

